# revision 42
# baseline (speedup 1.0000x reference)
"""CharLSTM tagger on 8 Trainium2 NeuronCores (Bass/Tile).

Strategy
--------
The word-level LSTM is a serial scan over N=8192 steps (hidden 512). The
recurrence contracts perturbations, so a zero-state warm-up of K=896 steps
reproduces the state to ~4e-4 relative. We split the sequence into 1024
chunks of 8 payload steps; each core batches its 128 chunks on the PE
partition dim and advances them together, so the whole scan is K+8=904
batched steps instead of 8192 serial ones.

Per core (SPMD, core j):
  ext range = global words [1024j-896, 1024j+1024) (clamped; pre-history
  rows get an i-gate kill pattern that freezes the state at zero).
  P2: char-LSTM over the ext words (one-hot matmul embedding lookup,
      batch=words on partitions).
  P3: xp = x @ Wih.T + b in fp16 (stored fp16 in DRAM to halve scan DMA).
      The word-embedding gather happens on the host (pure indexing); the
      gathered-and-transposed table arrives as a [512, EXT] fp16 input with
      zero-padded tail rows, and the char-LSTM h feeds a separate K=6
      matmul accumulation, so no big per-tile transposes are needed.
  P4: batched scan: pre = xp + h @ Whh.T on the PE with float32r operands
      (single-pass matmuls at near-fp32 precision; plain fp32 lowers to two
      hi/lo passes). All gate nonlinearities are a single Sigmoid per bank:
      tanh(x) = 2*sigmoid(2x) - 1, with the 2x folded into host-scaled
      g-columns and the state kept as h' = h/2 (Whh/out_W doubled on host).
      The cell update runs per 128-col bank (fused scalar_tensor_tensor
      ops) so ACT/DVE of bank b overlap the matmuls of bank b+1, and the
      per-bank PE transposes keep the tensor engine from HAM-rethrottling.
      Ping-pong transposed-state tiles decouple step t's writes from step
      t+1's reads. The last 8 steps stash logits; log-softmax runs once
      after the loop (single activation-table switch).
Host side does only slicing / transposition / permutation / gather / casts
/ scalar rescaling of the inputs (no model FLOPs).
"""
import numpy as np

# dims
N, LMAX, WED, WHD, CED, CHD, WV, CV, T = 8192, 16, 506, 512, 6, 6, 50000, 128, 64
NCORES = 8
PC = N // NCORES       # payload words per core
PAY = PC // 128        # payload steps per chunk (128 chunks batched per core)
K_WARM = 768           # zero-state warm-up steps

_CACHE = {}


def _gate_perm():
    p = np.zeros(4 * WHD, np.int64)
    for b in range(4):
        base = 512 * b
        p[base + 0:base + 128] = 0 * WHD + 128 * b + np.arange(128)      # i
        p[base + 128:base + 256] = 1 * WHD + 128 * b + np.arange(128)    # f
        p[base + 256:base + 384] = 3 * WHD + 128 * b + np.arange(128)    # o
        p[base + 384:base + 512] = 2 * WHD + 128 * b + np.arange(128)    # g
    return p


_CPERM = np.concatenate([np.arange(0, 6), np.arange(6, 12),
                         np.arange(18, 24), np.arange(12, 18)])


def _build(K):
    import concourse.bass as bass
    import concourse.bacc as bacc
    import concourse.tile as tile
    import concourse.mybir as mybir
    from concourse.masks import make_identity
    from contextlib import ExitStack

    dt = mybir.dt
    AF = mybir.ActivationFunctionType
    ALU = mybir.AluOpType

    EXT = K + PC
    WTN = EXT // 128
    LOOP = K // PAY
    NCHUNK = EXT // PAY
    NKILL = K // 128
    assert K % 128 == 0 and K % PAY == 0

    nc = bacc.Bacc("TRN2", target_bir_lowering=False, debug=False)
    f32, bf16, i32 = dt.float32, dt.bfloat16, dt.int32
    f32r, f16 = dt.float32r, dt.float16

    chars_in = nc.dram_tensor("chars_cm", [LMAX, EXT], f32, kind="ExternalInput").ap()
    cmask_in = nc.dram_tensor("cmask", [128, LMAX, WTN * CHD], dt.uint8,
                              kind="ExternalInput").ap()
    keep_in = nc.dram_tensor("keep", [128, WTN], f32, kind="ExternalInput").ap()
    kill30_in = nc.dram_tensor("kill30", [128, WTN], f32, kind="ExternalInput").ap()
    weT_in = nc.dram_tensor("weT_b", [512, EXT], f16, kind="ExternalInput").ap()
    cemb_in = nc.dram_tensor("char_emb", [CV, CED], f32, kind="ExternalInput").ap()
    cWihT_in = nc.dram_tensor("cWihT_p", [CED, 4 * CHD], f32, kind="ExternalInput").ap()
    cb_in = nc.dram_tensor("cb_p", [1, 4 * CHD], f32, kind="ExternalInput").ap()
    cWhhT_in = nc.dram_tensor("cWhhT_p", [CHD, 4 * CHD], f32, kind="ExternalInput").ap()
    Wr_in = nc.dram_tensor("Wr_b", [WHD, 4 * WHD], f16, kind="ExternalInput").ap()
    WihT_in = nc.dram_tensor("WihT_b", [WHD, 4 * WHD], f16,
                             kind="ExternalInput").ap()
    wb_in = nc.dram_tensor("wb_p", [1, 4 * WHD], f32, kind="ExternalInput").ap()
    outWT_in = nc.dram_tensor("outWT_b", [WHD, T], f16, kind="ExternalInput").ap()
    outb_in = nc.dram_tensor("outb", [1, T], f32, kind="ExternalInput").ap()

    out_dram = nc.dram_tensor("out", [PC, T], f32, kind="ExternalOutput").ap()
    xp_dram = nc.dram_tensor("xp_scratch", [EXT, 4 * WHD], f16).ap()

    # step-pair view: rows a*PAY + 2*p + s2 -> one DMA loads 2 steps
    xp_p = xp_dram.rearrange("(a p s2) g -> p a s2 g", p=PAY // 2, s2=2)
    out_r = out_dram.rearrange("(a s) t -> s a t", s=PAY)

    with tile.TileContext(nc) as tc, ExitStack() as ctx:
        pers = ctx.enter_context(tc.tile_pool(name="pers", bufs=1))

        ident = pers.tile([128, 128], f32)
        make_identity(nc, ident[:])
        identh = pers.tile([128, 128], f16)
        nc.vector.tensor_copy(identh[:], ident[:])
        ones1 = pers.tile([1, 128], f32)
        nc.vector.memset(ones1[:], 1.0)

        Wr_sb = [pers.tile([128, 4 * WHD], f16, tag=f"wr{c}", name=f"wr{c}")
                 for c in range(4)]
        for c in range(4):
            nc.sync.dma_start(Wr_sb[c][:], Wr_in[128 * c:128 * (c + 1), :])
        wb_sb = pers.tile([1, 4 * WHD], f32)
        nc.sync.dma_start(wb_sb[:], wb_in[:])
        outWT_sb = [pers.tile([128, T], f16, tag=f"ow{c}", name=f"ow{c}")
                    for c in range(4)]
        for c in range(4):
            nc.sync.dma_start(outWT_sb[c][:], outWT_in[128 * c:128 * (c + 1), :])
        outb_sb = pers.tile([1, T], f32)
        nc.sync.dma_start(outb_sb[:], outb_in[:])
        keep_sb = pers.tile([128, WTN], f32)
        nc.sync.dma_start(keep_sb[:], keep_in[:])
        kill30_sb = pers.tile([128, WTN], f32)
        nc.sync.dma_start(kill30_sb[:], kill30_in[:])
        cmask_sb = pers.tile([128, LMAX * WTN * CHD], dt.uint8)
        nc.sync.dma_start(cmask_sb[:], cmask_in.rearrange("p l m -> p (l m)"))
        cWhhT_sb = pers.tile([CHD, 4 * CHD], f32)
        nc.sync.dma_start(cWhhT_sb[:], cWhhT_in[:])

        cemb_sb = pers.tile([CV, CED], f32)
        nc.sync.dma_start(cemb_sb[:], cemb_in[:])
        cWihT_sb = pers.tile([CED, 4 * CHD], f32)
        nc.sync.dma_start(cWihT_sb[:], cWihT_in[:])
        cb_sb = pers.tile([1, 4 * CHD], f32)
        nc.sync.dma_start(cb_sb[:], cb_in[:])

        # Gathered word-emb (transposed, zero-padded to 512 rows), resident.
        weT_sb = [pers.tile([128, EXT], f16, tag=f"weT{c}", name=f"weT{c}")
                  for c in range(4)]
        for c in range(4):
            nc.sync.dma_start(weT_sb[c][:], weT_in[128 * c:128 * (c + 1), :])

        with tc.tile_pool(name="p0psum", bufs=2, space="PSUM") as p0psum, \
             tc.tile_pool(name="p0tmp", bufs=2) as p0tmp:
            cembT_ps = p0psum.tile([CED, CV], f32)
            nc.tensor.transpose(cembT_ps[:], cemb_sb[:], ident[:])
            cembT_sb = p0tmp.tile([CED, CV], f32)
            nc.vector.tensor_copy(cembT_sb[:], cembT_ps[:])
            E_ps = p0psum.tile([CV, 4 * CHD], f32)
            nc.tensor.matmul(E_ps[:], lhsT=cembT_sb[:], rhs=cWihT_sb[:],
                             start=True, stop=False, skip_group_check=True)
            nc.tensor.matmul(E_ps[:], lhsT=ones1[:], rhs=cb_sb[:],
                             start=False, stop=True, skip_group_check=True)
            E_bf = pers.tile([CV, 4 * CHD], bf16)
            nc.vector.tensor_copy(E_bf[:], E_ps[:])

        iota_i = pers.tile([128, 1], i32)
        nc.gpsimd.iota(iota_i[:], pattern=[[0, 1]], base=0, channel_multiplier=1)
        iota_f = pers.tile([128, 1], f32)
        nc.vector.tensor_copy(iota_f[:], iota_i[:])

        # ================= P2: char pipeline =================
        hc = pers.tile([128, WTN * CHD], f32)
        cc = pers.tile([128, WTN * CHD], f32)
        nc.vector.memset(hc[:], 0.0)
        nc.vector.memset(cc[:], 0.0)

        with tc.tile_pool(name="p2psum", bufs=2, space="PSUM") as p2psum, \
             tc.tile_pool(name="p2tmp", bufs=4) as p2tmp, \
             tc.tile_pool(name="oh", bufs=4) as ohpool, \
             tc.tile_pool(name="xproj_ps", bufs=2, space="PSUM") as xpjps:

            xproj_sb = pers.tile([128, WTN * LMAX * 4 * CHD], f32)
            xproj_v = xproj_sb[:].rearrange("p (t l g) -> p t l g", t=WTN, l=LMAX,
                                            g=4 * CHD)
            NW = WTN * 128
            blks = [(b0, min(b0 + 512, NW)) for b0 in range(0, NW, 512)]
            for l in range(LMAX):
                oh = ohpool.tile([128, WTN * 128], bf16, tag="oh")
                charsl = p2tmp.tile([1, WTN * 128], f32, tag="charsl")
                nc.sync.dma_start(charsl[:], chars_in[l:l + 1, :])
                for (b0, b1) in blks:
                    bc_ps = p2psum.tile([128, 512], f32, tag="bc", bufs=2, name="bc")
                    nc.tensor.matmul(bc_ps[:, 0:b1 - b0], lhsT=ones1[:],
                                     rhs=charsl[0:1, b0:b1], start=True, stop=True)
                    nc.vector.tensor_scalar(oh[:, b0:b1], bc_ps[:, 0:b1 - b0],
                                            iota_f[:, :1], None, op0=ALU.is_equal)
                for t in range(WTN):
                    xp_ps = xpjps.tile([128, 4 * CHD], f32, tag="xpj", bufs=2)
                    nc.tensor.matmul(xp_ps[:], lhsT=oh[:, 128 * t:128 * (t + 1)],
                                     rhs=E_bf[:], start=True, stop=True)
                    nc.scalar.copy(xproj_v[:, t, l, :], xp_ps[:])

            for l in range(LMAX):
                pre_ps = p2psum.tile([128, WTN * 4 * CHD], f32, tag="pre", bufs=2)
                for t in range(WTN):
                    tp = p2psum.tile([CHD, 128], f32, tag="ctp2", bufs=2, name="tp")
                    nc.tensor.transpose(tp[:], hc[:, CHD * t:CHD * (t + 1)], ident[:])
                    ht_ = p2tmp.tile([CHD, 128], f32, tag="hcTt", bufs=6, name="ht_")
                    nc.vector.tensor_copy(ht_[:], tp[:])
                    nc.tensor.matmul(pre_ps[:, 4 * CHD * t:4 * CHD * (t + 1)],
                                     lhsT=ht_[:], rhs=cWhhT_sb[:],
                                     start=True, stop=True)
                pre_v = pre_ps[:].rearrange("p (t g) -> p t g", t=WTN)
                gates = p2tmp.tile([128, WTN * 4 * CHD], f32, tag="gates")
                gates_v = gates[:].rearrange("p (t g) -> p t g", t=WTN)
                nc.vector.tensor_tensor(gates_v[:, :, :], pre_v[:, :, :],
                                        xproj_v[:, :, l, :], op=ALU.add)
                nc.scalar.activation(gates_v[:, :, 0:3 * CHD],
                                     gates_v[:, :, 0:3 * CHD], AF.Sigmoid)
                nc.scalar.activation(gates_v[:, :, 3 * CHD:],
                                     gates_v[:, :, 3 * CHD:], AF.Tanh)
                ig = p2tmp.tile([128, WTN * CHD], f32, tag="ig")
                ig_v = ig[:].rearrange("p (t h) -> p t h", t=WTN)
                cc_v = cc[:].rearrange("p (t h) -> p t h", t=WTN)
                nc.vector.tensor_tensor(ig_v[:, :, :], gates_v[:, :, 0:CHD],
                                        gates_v[:, :, 3 * CHD:4 * CHD], op=ALU.mult)
                fc = p2tmp.tile([128, WTN * CHD], f32, tag="fc")
                fc_v = fc[:].rearrange("p (t h) -> p t h", t=WTN)
                nc.vector.tensor_tensor(fc_v[:, :, :], gates_v[:, :, CHD:2 * CHD],
                                        cc_v[:, :, :], op=ALU.mult)
                cn = p2tmp.tile([128, WTN * CHD], f32, tag="cn")
                nc.vector.tensor_tensor(cn[:], fc[:], ig[:], op=ALU.add)
                thc = p2tmp.tile([128, WTN * CHD], f32, tag="thc")
                nc.scalar.activation(thc[:], cn[:], AF.Tanh)
                hn = p2tmp.tile([128, WTN * CHD], f32, tag="hn")
                hn_v = hn[:].rearrange("p (t h) -> p t h", t=WTN)
                nc.vector.tensor_tensor(hn_v[:, :, :], gates_v[:, :, 2 * CHD:3 * CHD],
                                        thc[:].rearrange("p (t h) -> p t h", t=WTN),
                                        op=ALU.mult)
                msk = cmask_sb[:, (l * WTN * CHD):((l + 1) * WTN * CHD)]
                nc.vector.copy_predicated(hc[:], msk, hn[:])
                nc.vector.copy_predicated(cc[:], msk, cn[:])

        # ================= P3: xp = x @ Wih.T + b  (bf16) =================
        # x = [we | h_char]: the we part is resident (weT_sb, rows 506-511
        # zero); the h_char part is added as a separate K=6 accumulation.
        WihT_sb = [pers.tile([128, 4 * WHD], f16, tag=f"wih{c}", name=f"wih{c}")
                   for c in range(4)]
        for c in range(4):
            nc.sync.dma_start(WihT_sb[c][:], WihT_in[128 * c:128 * (c + 1), :])
        WihT_tail = pers.tile([CHD, 4 * WHD], f16)
        nc.sync.dma_start(WihT_tail[:], WihT_in[WED:WED + CHD, :])

        with tc.tile_pool(name="p3psum", bufs=2, space="PSUM") as p3psum, \
             tc.tile_pool(name="p3tmp", bufs=3) as p3tmp, \
             tc.tile_pool(name="p3xps", bufs=2, space="PSUM") as p3xps:
            for t in range(WTN):
                tp = p3psum.tile([CHD, 128], f32, tag="hcT_ps", bufs=2)
                nc.tensor.transpose(tp[:], hc[:, CHD * t:CHD * (t + 1)], ident[:])
                hcT = p3tmp.tile([CHD, 128], f16, tag="hcT")
                nc.vector.tensor_copy(hcT[:], tp[:])
                for b in range(4):
                    xp_ps = p3xps.tile([128, 512], f32, tag="xp_ps", bufs=2)
                    for c in range(4):
                        nc.tensor.matmul(xp_ps[:],
                                         lhsT=weT_sb[c][:, 128 * t:128 * (t + 1)],
                                         rhs=WihT_sb[c][:, 512 * b:512 * (b + 1)],
                                         start=(c == 0), stop=False,
                                         skip_group_check=True)
                    nc.tensor.matmul(xp_ps[:], lhsT=hcT[:],
                                     rhs=WihT_tail[:, 512 * b:512 * (b + 1)],
                                     start=False, stop=False, skip_group_check=True)
                    nc.tensor.matmul(xp_ps[:], lhsT=ones1[:],
                                     rhs=wb_sb[:, 512 * b:512 * (b + 1)],
                                     start=False, stop=True, skip_group_check=True)
                    xp_sb = p3tmp.tile([128, 512], f16, tag="xp_sb")
                    if t < NKILL:
                        xp_f = p3tmp.tile([128, 512], f32, tag="xp_f")
                        nc.vector.tensor_scalar(xp_f[:], xp_ps[:],
                                                keep_sb[:, t:t + 1], None,
                                                op0=ALU.mult)
                        nc.vector.tensor_scalar(xp_f[:, 0:128], xp_f[:, 0:128],
                                                kill30_sb[:, t:t + 1], None,
                                                op0=ALU.add)
                        nc.vector.tensor_copy(xp_sb[:], xp_f[:])
                    else:
                        nc.vector.tensor_copy(xp_sb[:], xp_ps[:])
                    nc.sync.dma_start(
                        xp_dram[128 * t:128 * (t + 1), 512 * b:512 * (b + 1)],
                        xp_sb[:])

        # ================= P4: scan =================
        # Ping-pong transposed state: step parity alternates src/dst sets.
        hTA = [pers.tile([128, 128], f16, tag=f"hTA{c}", name=f"hTA{c}")
               for c in range(4)]
        hTB = [pers.tile([128, 128], f16, tag=f"hTB{c}", name=f"hTB{c}")
               for c in range(4)]
        for c in range(4):
            nc.vector.memset(hTA[c][:], 0.0)
        c_st = pers.tile([128, 512], f32)
        nc.vector.memset(c_st[:], 0.0)
        h_sb = pers.tile([128, 512], f32)
        gates_sc = pers.tile([128, 2048], f32)
        gv = gates_sc[:].rearrange("p (b x) -> p b x", b=4)
        hv = h_sb[:].rearrange("p (b x) -> p b x", b=4)
        lg_all = pers.tile([128, PAY * T], f32)

        scan_psum = ctx.enter_context(
            tc.tile_pool(name="scan_ps", bufs=1, space="PSUM"))
        tr_psum = ctx.enter_context(tc.tile_pool(name="tr_ps", bufs=2, space="PSUM"))
        xp_pool = ctx.enter_context(tc.tile_pool(name="xp_t", bufs=6))

        def load_pair(u, p):
            xp2 = xp_pool.tile([128, 4096], f16, tag="xp2")
            v = xp2[:].rearrange("a (s2 g) -> a s2 g", s2=2)
            if isinstance(u, int):
                nc.sync.dma_start(v, xp_p[p, u:u + 128, :, :])
            else:
                nc.sync.dma_start(v, xp_p[p, bass.ds(u, 128), :, :])
            return xp2

        def scan_step(u, s, xp2, hsrc, hdst, payload):
            xp_t = xp2[:, 2048 * (s % 2):2048 * (s % 2) + 2048]
            pre = [scan_psum.tile([128, 512], f32, tag=f"pre{b}", name=f"pre{b}")
                   for b in range(4)]
            ig = xp_pool.tile([128, 512], f32, tag="ig_sc")
            for b in range(4):
                nc.tensor.matmul(pre[b][:], lhsT=identh[:],
                                 rhs=xp_t[:, 512 * b:512 * (b + 1)],
                                 start=True, stop=False, skip_group_check=True)
                for c in range(4):
                    nc.tensor.matmul(pre[b][:], lhsT=hsrc[c][:],
                                     rhs=Wr_sb[c][:, 512 * b:512 * (b + 1)],
                                     start=False, stop=(c == 3),
                                     skip_group_check=True)
                # One sigmoid per bank: tanh(x) = 2*sigmoid(2x)-1 with the 2x
                # folded into the host-scaled g-columns, and the state kept
                # as h' = h/2 (compensated by doubling Whh/out_W on host).
                nc.scalar.activation(gv[:, b, 0:512], pre[b][:, 0:512],
                                     AF.Sigmoid)
                nc.vector.scalar_tensor_tensor(
                    ig[:, 128 * b:128 * (b + 1)], gv[:, b, 384:512], 0.5,
                    gv[:, b, 0:128], op0=ALU.subtract, op1=ALU.mult)
                nc.vector.tensor_tensor(c_st[:, 128 * b:128 * (b + 1)],
                                        gv[:, b, 128:256],
                                        c_st[:, 128 * b:128 * (b + 1)],
                                        op=ALU.mult)
                nc.vector.scalar_tensor_tensor(
                    c_st[:, 128 * b:128 * (b + 1)],
                    ig[:, 128 * b:128 * (b + 1)], 2.0,
                    c_st[:, 128 * b:128 * (b + 1)],
                    op0=ALU.mult, op1=ALU.add)
                # per-bank tail: sigmoid(2c), h', transpose — keeps PE fed
                # every ~1.5us so HAM stays at full clock
                tc_t = xp_pool.tile([128, 128], f32, tag=f"tc_sc{b}",
                                    name=f"tc_sc{b}")
                nc.scalar.activation(tc_t[:], c_st[:, 128 * b:128 * (b + 1)],
                                     AF.Sigmoid, scale=2.0)
                nc.vector.scalar_tensor_tensor(
                    h_sb[:, 128 * b:128 * (b + 1)], tc_t[:], 0.5,
                    gv[:, b, 256:384], op0=ALU.subtract, op1=ALU.mult)
            # transposes issued after ALL bank matmul groups so the PE's
            # in-order stream never stalls between banks; each bank's h'
            # is ready by the time the PE drains the later banks' matmuls
            for b in range(4):
                tr = tr_psum.tile([128, 128], f32, tag="tr", name="tr")
                nc.tensor.transpose(tr[:], h_sb[:, 128 * b:128 * (b + 1)], ident[:])
                nc.vector.tensor_copy(hdst[b][:], tr[:])
            if payload:
                lg_ps = tr_psum.tile([128, T], f32, tag="lg", name="lg")
                for c in range(4):
                    nc.tensor.matmul(lg_ps[:], lhsT=hdst[c][:],
                                     rhs=outWT_sb[c][:],
                                     start=(c == 0), stop=False,
                                     skip_group_check=True)
                nc.tensor.matmul(lg_ps[:], lhsT=ones1[:], rhs=outb_sb[:],
                                 start=False, stop=True, skip_group_check=True)
                nc.vector.tensor_copy(lg_all[:, T * s:T * (s + 1)], lg_ps[:])

        def ab(s):
            return (hTA, hTB) if s % 2 == 0 else (hTB, hTA)

        assert PAY % 2 == 0
        if LOOP > 0:
            with tc.For_i(0, LOOP, 1,
                          hint_engines=(mybir.EngineType.PE,),
                          staggered_reset=True) as u:
                for s in range(PAY):
                    if s % 2 == 0:
                        xp2 = load_pair(u, s // 2)
                    src, dst = ab(s)
                    scan_step(u, s, xp2, src, dst, payload=False)
        for s in range(PAY):
            if s % 2 == 0:
                xp2 = load_pair(LOOP, s // 2)
            src, dst = ab(s)
            scan_step(LOOP, s, xp2, src, dst, payload=True)

        # log-softmax over the stashed logits (one table-switch).
        for s in range(PAY):
            lg = lg_all[:, T * s:T * (s + 1)]
            mx = xp_pool.tile([128, 1], f32, tag="mx")
            nc.vector.tensor_reduce(mx[:], lg, axis=mybir.AxisListType.X,
                                    op=ALU.max)
            nmx = xp_pool.tile([128, 1], f32, tag="nmx")
            nc.vector.tensor_scalar_mul(nmx[:], mx[:], -1.0)
            ex = xp_pool.tile([128, T], f32, tag="ex")
            sume = xp_pool.tile([128, 1], f32, tag="sume")
            nc.scalar.activation(ex[:], lg, AF.Exp, bias=nmx[:, :1],
                                 scale=1.0, accum_out=sume[:, :1])
            lse = xp_pool.tile([128, 1], f32, tag="lse")
            nc.scalar.activation(lse[:], sume[:], AF.Ln)
            off = xp_pool.tile([128, 1], f32, tag="off")
            nc.vector.tensor_tensor(off[:], mx[:], lse[:], op=ALU.add)
            ls = xp_pool.tile([128, T], f32, tag="ls")
            nc.vector.tensor_scalar(ls[:], lg, off[:, :1], None,
                                    op0=ALU.subtract)
            nc.sync.dma_start(out_r[s], ls[:])

    nc.compile()
    return nc


def _host_prep(inp, K):
    EXT = K + PC
    WTN = EXT // 128
    perm = _gate_perm()
    # scale factors for the all-sigmoid reformulation: state is h' = h/2
    # (so Whh doubles), and g-columns get an extra 2x for tanh(x)=2sig(2x)-1
    gsc = np.ones(4 * WHD, np.float32)
    for _b in range(4):
        gsc[512 * _b + 384:512 * _b + 512] = 2.0
    Wr_b = np.ascontiguousarray(
        np.asarray(inp["w_Whh"], np.float32).T[:, perm]).astype(np.float32)
    Wr_b = Wr_b * (2.0 * gsc)[None, :]
    WihT_p = np.ascontiguousarray(np.asarray(inp["w_Wih"], np.float32).T[:, perm])
    WihT_p = WihT_p * gsc[None, :]
    wb_p = (np.ascontiguousarray(np.asarray(inp["w_b"], np.float32)[perm])
            * gsc)[None, :]
    outWT = np.ascontiguousarray(np.asarray(inp["out_W"], np.float32).T) * 2.0
    outb = np.asarray(inp["out_b"], np.float32)[None, :]
    cWihT_p = np.ascontiguousarray(np.asarray(inp["c_Wih"], np.float32).T[:, _CPERM])
    cWhhT_p = np.ascontiguousarray(np.asarray(inp["c_Whh"], np.float32).T[:, _CPERM])
    cb_p = np.ascontiguousarray(np.asarray(inp["c_b"], np.float32)[_CPERM])[None, :]
    word_emb = np.asarray(inp["word_emb"], np.float32)
    char_emb = np.ascontiguousarray(np.asarray(inp["char_emb"], np.float32))

    Wr_b16 = Wr_b.astype(np.float16)
    WihT_b16 = WihT_p.astype(np.float16)
    outWT_b16 = outWT.astype(np.float16)

    words = np.asarray(inp["sentence_words"])
    chars = np.asarray(inp["sentence_characters"])
    lens = np.asarray(inp["char_lengths"])

    in_maps = []
    for j in range(NCORES):
        g0 = PC * j - K
        gidx = np.arange(g0, g0 + EXT)
        valid = gidx >= 0
        gc = np.clip(gidx, 0, N - 1)
        chars_j = chars[gc].astype(np.float32)
        lens_j = lens[gc].astype(np.int64)
        # gathered + transposed word embeddings, padded to 512 rows
        weT = np.zeros((512, EXT), dtype=np.float16)
        weT[:WED, :] = word_emb[words[gc]].T.astype(np.float16)
        widx = (np.arange(WTN)[None, :] * 128 + np.arange(128)[:, None])
        lens_pt = lens_j[widx]
        lmask = (np.arange(LMAX)[None, :, None] < lens_pt[:, None, :])
        cmask = np.repeat(lmask, CHD, axis=2).astype(np.uint8)
        keep = valid[widx].astype(np.float32)
        kill30 = ((1.0 - keep) * (-30.0)).astype(np.float32)
        in_maps.append({
            "chars_cm": np.ascontiguousarray(chars_j.T),
            "cmask": np.ascontiguousarray(cmask),
            "keep": keep, "kill30": kill30,
            "weT_b": weT, "char_emb": char_emb,
            "cWihT_p": cWihT_p, "cb_p": cb_p, "cWhhT_p": cWhhT_p,
            "Wr_b": Wr_b16, "WihT_b": WihT_b16, "wb_p": wb_p,
            "outWT_b": outWT_b16, "outb": outb,
        })
    return in_maps


def kernel(**inputs):
    from concourse.bass_utils import run_bass_kernel_spmd

    if "nc" not in _CACHE:
        _CACHE["nc"] = _build(K_WARM)
    nc = _CACHE["nc"]
    in_maps = _host_prep(inputs, K_WARM)
    res = run_bass_kernel_spmd(nc, in_maps, list(range(NCORES)))
    out = np.concatenate([res.results[j]["out"] for j in range(NCORES)], axis=0)
    return out.astype(np.float32)
